# revision 1
# baseline (speedup 1.0000x reference)
"""Trainium2 Bass kernel for nn_Attention_34376918237341.

Dense causal GQA attention block (QKV proj -> QK RMSNorm -> RoPE + per-head
q gain -> causal SDPA -> out proj), B=4 T=2048 D=2048 H=16 KVH=4 HD=128, fp32.

Sharding across 8 NeuronCores: 4-way data-parallel over batch x 2-way
tensor-parallel over heads (8 q heads / 2 kv heads per core). Each core
computes a partial projection output; the host sums the two head-group
partials per batch.

Device pipeline per core (single Bass program, SPMD over 8 cores):
  A) QKV projections from host-pre-transposed operands, fused RMSNorm (via
     Square+accum on ScalarE, rsqrt = exp(-0.5*ln) to stay in one ACT table
     set) and RoPE (host-baked cos/sinflip tables with q_gain folded in),
     PE-transpose of q/k into SBUF-resident [HD, T] tensors (packed PSUM
     tile, two wide DVE evacuation copies).
  B) Per (q block, kv head, q head): S^T = K Q^T on PE, exp on ScalarE
     (no-max softmax: |S| <= gain*sqrt(HD)+eps so exp cannot overflow;
     fully-causal-masked columns of diagonal chunks are skipped outright,
     the remaining 128x128 triangular strip masked via gpsimd
     affine_select), softmax denominator l via a ones-vector matmul, y^T
     accumulated on PE with V as the stationary operand; 1/l = exp(-ln l)
     batched over the 4 q heads, broadcast across partitions with a K=1
     matmul, and folded into the y^T normalization multiply.
  C) Output projection from SBUF-resident y^T and Wproj^T.

Matmul operand dtype via BASS_MM env: "bf16" (default; fp32 PSUM
accumulation, end-to-end rel err ~3e-3) or "f32" (bit-accurate ~5e-6,
~3x slower on the PE).
"""

import math
import os
import sys

import numpy as np

sys.path.insert(0, "/opt/trn_rl_repo")

import concourse.bass as bass  # noqa: E402
import concourse.tile as tile  # noqa: E402
from concourse import bacc, mybir  # noqa: E402
from concourse.bass_utils import run_bass_kernel_spmd  # noqa: E402
from concourse.masks import make_identity  # noqa: E402

F32 = mybir.dt.float32
AF = mybir.ActivationFunctionType
ALU = mybir.AluOpType

B, T, D = 4, 2048, 2048
H, KVH, HD = 16, 4, 128
GH, GKV = 8, 2          # q heads / kv heads per core (2-way TP)
GD, GKD = GH * HD, GKV * HD   # 1024 / 256
BASE = 10000.0
EPS = 1.1920928955078125e-07
NCORES = 8
P = 128
NT = T // P             # 16 row tiles
NKC = D // P            # 16 contraction chunks for QKV
QBLK = 512              # q-block width in phase B
NB = T // QBLK          # 4 q blocks
SCALE = 1.0 / math.sqrt(HD)

MM_MODE = os.environ.get("BASS_MM", "bf16")  # "f32" | "f32r" | "bf16"
MMDT = {"f32": mybir.dt.float32, "f32r": mybir.dt.float32r,
        "bf16": mybir.dt.bfloat16}[MM_MODE]

_CACHE = {}


def _mm_ap(ap):
    return ap


def _build_program():
    nc = bacc.Bacc(
        "TRN2", target_bir_lowering=False, debug=False, num_devices=NCORES
    )

    # ---- DRAM I/O ----
    xT = nc.dram_tensor("xT", [D, T], MMDT, kind="ExternalInput").ap()
    wqT = nc.dram_tensor("wqT", [D, GD], MMDT, kind="ExternalInput").ap()
    wkvT = nc.dram_tensor("wkvT", [D, 2 * GKD], MMDT, kind="ExternalInput").ap()
    wpT = nc.dram_tensor("wpT", [GD, D], MMDT, kind="ExternalInput").ap()
    cosq = nc.dram_tensor("cosq", [T, GD], F32, kind="ExternalInput").ap()
    sinq = nc.dram_tensor("sinq", [T, GD], F32, kind="ExternalInput").ap()
    cosk = nc.dram_tensor("cosk", [T, GKD], F32, kind="ExternalInput").ap()
    sink = nc.dram_tensor("sink", [T, GKD], F32, kind="ExternalInput").ap()
    out = nc.dram_tensor("out", [T, D], F32, kind="ExternalOutput").ap()


    xT_v = xT.rearrange("(ko p) t -> p ko t", p=P)
    wqT_v = wqT.rearrange("(ko p) m -> p ko m", p=P)
    wkvT_v = wkvT.rearrange("(ko p) m -> p ko m", p=P)
    wpT_v = wpT.rearrange("(ko p) m -> p ko m", p=P)

    with tile.TileContext(nc) as tc:
        with (
            tc.tile_pool(name="const", bufs=1) as constp,
            tc.tile_pool(name="resident", bufs=1) as respool,
        ):
            ident = constp.tile([P, P], MMDT)
            make_identity(nc, ident)
            ones_col = constp.tile([P, 1], MMDT)   # lhsT for l row-sums
            nc.vector.memset(ones_col[:], 1.0)
            ones_row = constp.tile([1, P], MMDT)   # lhsT for 1/l broadcast
            nc.vector.memset(ones_row[:], 1.0)
            eps_col = constp.tile([P, 1], F32)    # rmsnorm eps as ACT bias
            nc.vector.memset(eps_col[:], EPS)
            zero_col = constp.tile([P, 1], F32)   # zero bias for Ln/Exp
            nc.vector.memset(zero_col[:], 0.0)

            qT_all = respool.tile([P, GH, T], MMDT)
            kT_all = respool.tile([P, GKV, T], MMDT)
            v_all = respool.tile([P, NT, GKV, HD], MMDT)

            # ================= Phase A: QKV + norm + rope + transpose ====
            with (
                tc.tile_pool(name="wqkv", bufs=1) as wpool,
                tc.tile_pool(name="pa_sb", bufs=2) as sb,
                tc.tile_pool(name="pa_ps", bufs=2, space="PSUM") as ps,
                tc.tile_pool(name="pa_ps1", bufs=1, space="PSUM") as ps1a,
            ):
                xt0 = sb.tile([P, NKC, P], MMDT, tag="xt")
                # first k-chunk separately: the first matmul gates on 32KB,
                # not the full tile, while the 6MB weight preload streams
                nc.sync.dma_start(xt0[:, 0:1, :], xT_v[:, 0:1, bass.ts(0, P)])
                nc.sync.dma_start(xt0[:, 1:NKC, :],
                                  xT_v[:, 1:NKC, bass.ts(0, P)])
                cq0 = sb.tile([P, GD], F32, tag="cq")
                nc.sync.dma_start(cq0[:], cosq[bass.ts(0, P), :])
                sq0 = sb.tile([P, GD], F32, tag="sq")
                nc.sync.dma_start(sq0[:], sinq[bass.ts(0, P), :])
                ck0 = sb.tile([P, GKD], F32, tag="ck")
                nc.sync.dma_start(ck0[:], cosk[bass.ts(0, P), :])
                sk0 = sb.tile([P, GKD], F32, tag="sk")
                nc.sync.dma_start(sk0[:], sink[bass.ts(0, P), :])
                wq_sb = wpool.tile([P, NKC, GD], MMDT)
                wkv_sb = wpool.tile([P, NKC, 2 * GKD], MMDT)
                for kc in range(NKC):
                    nc.sync.dma_start(wq_sb[:, kc, :], wqT_v[:, kc, :])
                    nc.sync.dma_start(wkv_sb[:, kc, :], wkvT_v[:, kc, :])

                for i in range(NT):
                    tsl = bass.ts(i, P)
                    if i == 0:
                        xt, cq, sq, ck, sk = xt0, cq0, sq0, ck0, sk0
                    else:
                        xt = sb.tile([P, NKC, P], MMDT, tag="xt")
                        nc.sync.dma_start(xt[:], xT_v[:, :, tsl])
                        cq = sb.tile([P, GD], F32, tag="cq")
                        nc.sync.dma_start(cq[:], cosq[tsl, :])
                        sq = sb.tile([P, GD], F32, tag="sq")
                        nc.sync.dma_start(sq[:], sinq[tsl, :])
                        ck = sb.tile([P, GKD], F32, tag="ck")
                        nc.sync.dma_start(ck[:], cosk[tsl, :])
                        sk = sb.tile([P, GKD], F32, tag="sk")
                        nc.sync.dma_start(sk[:], sink[tsl, :])

                    q_ps = ps.tile([P, GD], F32, tag="qps")
                    kv_ps = ps.tile([P, 2 * GKD], F32, tag="kvps")
                    k_ps = kv_ps[:, 0:GKD]
                    v_ps = kv_ps[:, GKD:2 * GKD]
                    for kc in range(NKC):
                        st, sp = kc == 0, kc == NKC - 1
                        lx = _mm_ap(xt[:, kc, :])
                        nc.tensor.matmul(q_ps[:, 0:512], lx,
                                         _mm_ap(wq_sb[:, kc, 0:512]),
                                         start=st, stop=sp)
                        nc.tensor.matmul(q_ps[:, 512:1024], lx,
                                         _mm_ap(wq_sb[:, kc, 512:1024]),
                                         start=st, stop=sp)
                        nc.tensor.matmul(kv_ps[:], lx,
                                         _mm_ap(wkv_sb[:, kc, :]),
                                         start=st, stop=sp)

                    # ---- sum of squares per head (ScalarE), rstd ----
                    ssq = sb.tile([P, GH + GKV], F32, tag="ssq")
                    scr = sb.tile([P, P], F32, tag="scr")
                    for h in range(GH):
                        nc.scalar.activation(scr[:], q_ps[:, h * HD:(h + 1) * HD],
                                             AF.Square,
                                             accum_out=ssq[:, h:h + 1])
                    for h in range(GKV):
                        nc.scalar.activation(scr[:], k_ps[:, h * HD:(h + 1) * HD],
                                             AF.Square,
                                             accum_out=ssq[:, GH + h:GH + h + 1])
                    lns = sb.tile([P, GH + GKV], F32, tag="lns")
                    nc.scalar.activation(lns[:], ssq[:], AF.Ln,
                                         scale=1.0 / HD, bias=eps_col[:])
                    rstd = sb.tile([P, GH + GKV], F32, tag="rstd")
                    nc.scalar.activation(rstd[:], lns[:], AF.Exp, scale=-0.5,
                                         bias=zero_col[:])

                    # ---- rope: (q*cos + shift(q)*sinflip) * rstd ----
                    def rope(z_ps, ct, st_, rs, nh, tag):
                        w = nh * HD
                        t1 = sb.tile([P, w], F32, tag=tag + "t1")
                        nc.vector.tensor_tensor(t1[:], z_ps[:, :w], ct[:, :w],
                                                ALU.mult)
                        t2 = sb.tile([P, w], F32, tag=tag + "t2")
                        z3 = z_ps[:, :w].rearrange("p (h d) -> p h d", h=nh)
                        t23 = t2[:].rearrange("p (h d) -> p h d", h=nh)
                        st3 = st_[:, :w].rearrange("p (h d) -> p h d", h=nh)
                        nc.vector.tensor_tensor(t23[:, :, 0:64],
                                                z3[:, :, 64:128],
                                                st3[:, :, 0:64], ALU.mult)
                        nc.vector.tensor_tensor(t23[:, :, 64:128],
                                                z3[:, :, 0:64],
                                                st3[:, :, 64:128], ALU.mult)
                        nc.vector.tensor_tensor(t1[:], t1[:], t2[:], ALU.add)
                        zf = sb.tile([P, w], MMDT, tag=tag + "zf")
                        zf3 = zf[:].rearrange("p (h d) -> p h d", h=nh)
                        t13 = t1[:].rearrange("p (h d) -> p h d", h=nh)
                        nc.vector.tensor_tensor(
                            zf3, t13,
                            rs[:, :, None].to_broadcast((P, nh, HD)), ALU.mult)
                        return zf

                    qf = rope(q_ps, cq, sq, rstd[:, 0:GH], GH, "q")
                    kf = rope(k_ps, ck, sk, rstd[:, GH:GH + GKV], GKV, "k")

                    # ---- PE transpose to [HD, T] layout, SBUF-resident.
                    # All 10 head transposes land in one packed PSUM tile
                    # (bf16: 10*256B fits 2 banks), evacuated by 2 wide DVE
                    # copies -- avoids a per-head PE<->DVE ping-pong.
                    tp = ps1a.tile([P, GH + GKV, P], MMDT, tag="tp")
                    for h in range(GH):
                        nc.tensor.transpose(tp[:, h, :],
                                            qf[:, h * HD:(h + 1) * HD],
                                            ident[:])
                    for h in range(GKV):
                        nc.tensor.transpose(tp[:, GH + h, :],
                                            kf[:, h * HD:(h + 1) * HD],
                                            ident[:])
                    nc.vector.tensor_copy(qT_all[:, :, tsl], tp[:, 0:GH, :])
                    nc.vector.tensor_copy(kT_all[:, :, tsl],
                                          tp[:, GH:GH + GKV, :])
                    nc.vector.tensor_copy(
                        v_all[:, i, :, :],
                        v_ps.rearrange("p (h d) -> p h d", h=GKV))


            # ================= Phase B: attention ========================
            with (
                tc.tile_pool(name="yall", bufs=1) as ypool,
                tc.tile_pool(name="pb_sb", bufs=2) as sb,
                tc.tile_pool(name="pb_pt", bufs=4) as ptp,
            ):
                yT_all = ypool.tile([P, GH, T], MMDT)
                wp_sb = ypool.tile([P, GH, D], MMDT)
                nc.sync.dma_start(wp_sb[:], wpT_v)
                with (
                    tc.tile_pool(name="pb_ps", bufs=2, space="PSUM") as ps,
                    tc.tile_pool(name="pb_ps1", bufs=2, space="PSUM") as ps1,
                    tc.tile_pool(name="pb_li", bufs=1, space="PSUM") as psli,
                    tc.tile_pool(name="pc_ps", bufs=1, space="PSUM") as cps,
                ):
                 def proj_tile(i):
                    # interleaved output projection for one 128-row tile,
                    # split into two 1024-wide halves (2 PSUM banks each)
                    tsl = bass.ts(i, P)
                    for nh in range(4):
                        o_ps = cps.tile([P, D // 4], F32, tag="ops")
                        for kc in range(GH):
                            nc.tensor.matmul(o_ps[:],
                                             _mm_ap(yT_all[:, kc, tsl]),
                                             _mm_ap(wp_sb[:, kc,
                                                    bass.ts(nh, 512)]),
                                             start=kc == 0, stop=kc == GH - 1)
                        o_sb = sb.tile([P, D // 4], F32, tag="osb")
                        nc.vector.tensor_copy(o_sb[:], o_ps[:])
                        nc.sync.dma_start(
                            out[tsl, bass.ts(nh, 512)], o_sb[:])

                 def emit_tail(bt, kht, l4, y4, bsl_t):
                    lnl = sb.tile([1, 4 * QBLK], F32, tag="lnl")
                    nc.scalar.activation(lnl[:], l4[:], AF.Ln,
                                         bias=zero_col[:1])
                    linv = sb.tile([1, 4 * QBLK], MMDT, tag="linv")
                    nc.scalar.activation(linv[:], lnl[:], AF.Exp,
                                         scale=-1.0, bias=zero_col[:1])
                    for hi, h in enumerate(range(kht * 4, kht * 4 + 4)):
                        li_ps = psli.tile([P, QBLK], F32, tag="lips")
                        nc.tensor.matmul(
                            li_ps[:], _mm_ap(ones_row[:]),
                            _mm_ap(linv[:, hi * QBLK:(hi + 1) * QBLK]),
                            start=True, stop=True)
                        nc.vector.tensor_tensor(yT_all[:, h, bsl_t],
                                                y4[:, hi, :],
                                                li_ps[:], ALU.mult)

                 pending = None
                 pending_proj = None
                 for b in range(NB):
                    nch = (b + 1) * (QBLK // P)
                    bsl = bass.ds(b * QBLK, QBLK)
                    for kh in range(GKV):
                        kt_blk = kT_all[:, kh, :]
                        l4 = sb.tile([1, 4 * QBLK], F32, tag="l4")
                        y4 = sb.tile([P, 4, QBLK], F32, tag="y4")
                        for hi, h in enumerate(range(kh * 4, kh * 4 + 4)):
                            qt_blk = qT_all[:, h, bsl]
                            l_ps = ps1.tile([1, QBLK], F32, tag="lps")
                            y_ps = ps1.tile([P, QBLK], F32, tag="yps")
                            for c in range(nch):
                                # columns x < x0 of this chunk are fully
                                # masked by causality; skip them entirely
                                x0 = max(0, (c - 4 * b) * P)
                                w = QBLK - x0
                                st_ps = ps.tile([P, QBLK], F32, tag="stps")
                                nc.tensor.matmul(
                                    st_ps[:, x0:QBLK],
                                    _mm_ap(kt_blk[:, c * P:(c + 1) * P]),
                                    _mm_ap(qt_blk[:, x0:QBLK]),
                                    start=True, stop=True)
                                pt = ptp.tile([P, QBLK], MMDT, tag="pt")
                                nc.scalar.activation(pt[:, x0:QBLK],
                                                     st_ps[:, x0:QBLK],
                                                     AF.Exp, scale=SCALE,
                                                     bias=zero_col[:])
                                if c >= 4 * b:
                                    # triangular strip: keep where x - p >= 0
                                    nc.gpsimd.affine_select(
                                        out=pt[:, x0:x0 + P],
                                        in_=pt[:, x0:x0 + P],
                                        compare_op=ALU.is_ge, fill=0.0,
                                        base=0, channel_multiplier=-1,
                                        pattern=[[1, P]])
                                stt, spp = c == 0, c == nch - 1
                                nc.tensor.matmul(l_ps[:, x0:QBLK],
                                                 _mm_ap(ones_col[:]),
                                                 _mm_ap(pt[:, x0:QBLK]),
                                                 start=stt, stop=spp)
                                nc.tensor.matmul(y_ps[:, x0:QBLK],
                                                 _mm_ap(v_all[:, c, kh, :]),
                                                 _mm_ap(pt[:, x0:QBLK]),
                                                 start=stt, stop=spp)
                            # evacuate unnormalized y and l; batch the
                            # 1/l = exp(-ln l) over the 4 heads afterwards
                            nc.vector.tensor_copy(
                                l4[:, hi * QBLK:(hi + 1) * QBLK], l_ps[:])
                            nc.vector.tensor_copy(y4[:, hi, :], y_ps[:])
                        if pending_proj is not None:
                            for i in range(4 * pending_proj,
                                           4 * pending_proj + 4):
                                proj_tile(i)
                            pending_proj = None
                        if pending is not None:
                            emit_tail(*pending)
                            if pending[1] == 1:
                                pending_proj = pending[0]
                        pending = (b, kh, l4, y4, bsl)

                 if pending_proj is not None:
                     for i in range(4 * pending_proj, 4 * pending_proj + 4):
                         proj_tile(i)
                 if pending is not None:
                     emit_tail(*pending)
                     if pending[1] == 1:
                         for i in range(4 * pending[0], 4 * pending[0] + 4):
                             proj_tile(i)

    nc.compile()
    return nc


def _np_mmdt():
    if MM_MODE == "bf16":
        import ml_dtypes
        return ml_dtypes.bfloat16
    return np.float32


def _host_prep(x, Wq, Wk, Wv, Wproj, q_gain):
    """Build the 8 per-core input maps."""
    mdt = _np_mmdt()
    t = np.arange(T, dtype=np.float64)
    inv_freq = 1.0 / (BASE ** (np.arange(0, HD, 2, dtype=np.float64) / HD))
    freqs = np.outer(t, inv_freq)
    emb = np.concatenate([freqs, freqs], axis=-1)
    cos = np.cos(emb).astype(np.float32)
    sin = np.sin(emb).astype(np.float32)
    sinflip = np.concatenate([-sin[:, :64], sin[:, :64]], axis=-1)

    cosk = np.ascontiguousarray(np.tile(cos, (1, GKV)))
    sink = np.ascontiguousarray(np.tile(sinflip, (1, GKV)))

    in_maps = []
    for c in range(NCORES):
        b, g = c // 2, c % 2
        gain = q_gain[g * GH:(g + 1) * GH].astype(np.float32)
        gexp = np.repeat(gain, HD)[None, :]
        in_maps.append({
            "xT": np.ascontiguousarray(x[b].T).astype(mdt),
            "wqT": np.ascontiguousarray(Wq[g * GD:(g + 1) * GD, :].T).astype(mdt),
            "wkvT": np.ascontiguousarray(np.concatenate(
                [Wk[g * GKD:(g + 1) * GKD, :].T,
                 Wv[g * GKD:(g + 1) * GKD, :].T], axis=1)).astype(mdt),
            "wpT": np.ascontiguousarray(Wproj[:, g * GD:(g + 1) * GD].T).astype(mdt),
            "cosq": np.ascontiguousarray(np.tile(cos, (1, GH)) * gexp),
            "sinq": np.ascontiguousarray(np.tile(sinflip, (1, GH)) * gexp),
            "cosk": cosk,
            "sink": sink,
        })
    return in_maps


def run(x, Wq, Wk, Wv, Wproj, q_gain, trace=False):
    if "nc" not in _CACHE:
        _CACHE["nc"] = _build_program()
    nc = _CACHE["nc"]
    in_maps = _host_prep(
        np.asarray(x, np.float32), np.asarray(Wq, np.float32),
        np.asarray(Wk, np.float32), np.asarray(Wv, np.float32),
        np.asarray(Wproj, np.float32), np.asarray(q_gain, np.float32))
    try:
        res = run_bass_kernel_spmd(nc, in_maps, list(range(NCORES)),
                                   trace=trace)
    except ModuleNotFoundError:
        res = run_bass_kernel_spmd(nc, in_maps, list(range(NCORES)),
                                   trace=False)
    outs = np.zeros((B, T, D), np.float32)
    for c in range(NCORES):
        outs[c // 2] += res.results[c]["out"]
    return outs, res.exec_time_ns


def kernel(**inputs):
    out, _ = run(inputs["x"], inputs["Wq"], inputs["Wk"], inputs["Wv"],
                 inputs["Wproj"], inputs["q_gain"])
    return out



# revision 12
# speedup vs baseline: 1.1794x; 1.1794x over previous
"""Trainium2 Bass kernel for nn_Attention_34376918237341.

Dense causal GQA attention block (QKV proj -> QK RMSNorm -> RoPE + per-head
q gain -> causal SDPA -> out proj), B=4 T=2048 D=2048 H=16 KVH=4 HD=128, fp32.

Sharding across 8 NeuronCores: 4-way data-parallel over batch x 2-way
tensor-parallel over heads (8 q heads / 2 kv heads per core). Each core
computes a partial projection output; the host sums the two head-group
partials per batch.

Device pipeline per core (single Bass program, SPMD over 8 cores):
  A) QKV projections from host-pre-transposed operands; RMSNorm rstd via
     Square+accum (ScalarE) -> (ssq/HD+eps) -> DVE reciprocal -> ScalarE
     Sqrt, with the per-head q gain folded in via a g^2 constant tile (so
     the RoPE cos/sin tables are a single shared [T, HD] pair instead of
     per-head tiled copies); RoPE from the shared tables with stride-0
     head broadcast; PE-transpose of q/k into SBUF-resident [HD, T]
     tensors. Wproj streams into SBUF during phase A.
     Activation-function tables: phase A uses only Square/Sqrt (one set),
     phase B only Exp -- 2-3 table loads total instead of ~49.
  B) Per (q block, kv head, q head): S^T = K Q^T on PE, exp on ScalarE
     (no-max softmax: |S| <= gain*sqrt(HD)+eps so exp cannot overflow;
     fully-causal-masked columns of diagonal chunks are skipped outright,
     the remaining 128x128 triangular strip masked via gpsimd
     affine_select), y^T accumulated on PE with V as the stationary
     operand. Softmax denominator l via *transposed* matmuls (pt chunk as
     the stationary operand, ones column as the 1-wide moving operand:
     output free size 1, so the PE streaming cost is ~zero), accumulated
     as [x, 1] PSUM columns; 1/l via DVE reciprocal, PE-transposed to row
     layout and broadcast across partitions with K=1 matmuls, folded into
     the y^T normalization multiply.
  C) Output projection from SBUF-resident y^T and Wproj^T (interleaved
     with phase B per query block, double-buffered PSUM).

Matmul operand dtype via BASS_MM env: "bf16" (default; fp32 PSUM
accumulation, end-to-end rel err ~3e-3) or "f32" (bit-accurate ~5e-6,
~3x slower on the PE).
"""

import math
import os
import sys

import numpy as np

sys.path.insert(0, "/opt/trn_rl_repo")

import concourse.bass as bass  # noqa: E402
import concourse.tile as tile  # noqa: E402
from concourse import bacc, mybir  # noqa: E402
from concourse.bass_utils import run_bass_kernel_spmd  # noqa: E402
from concourse.masks import make_identity  # noqa: E402

F32 = mybir.dt.float32
AF = mybir.ActivationFunctionType
ALU = mybir.AluOpType

B, T, D = 4, 2048, 2048
H, KVH, HD = 16, 4, 128
GH, GKV = 8, 2          # q heads / kv heads per core (2-way TP)
GD, GKD = GH * HD, GKV * HD   # 1024 / 256
NH = GH + GKV           # rmsnorm'd heads per core
BASE = 10000.0
EPS = 1.1920928955078125e-07
NCORES = 8
P = 128
NT = T // P             # 16 row tiles
NKC = D // P            # 16 contraction chunks for QKV
QBLK = 512              # q-block width in phase B
NB = T // QBLK          # 4 q blocks
NJ = QBLK // P          # 4 x-chunks per q block
SCALE = 1.0 / math.sqrt(HD)

MM_MODE = os.environ.get("BASS_MM", "bf16")  # "f32" | "f32r" | "bf16"
MMDT = {"f32": mybir.dt.float32, "f32r": mybir.dt.float32r,
        "bf16": mybir.dt.bfloat16}[MM_MODE]

_CACHE = {}


def _mm_ap(ap):
    return ap


def _build_program():
    nc = bacc.Bacc(
        "TRN2", target_bir_lowering=False, debug=False, num_devices=NCORES
    )

    # ---- DRAM I/O ----
    xT = nc.dram_tensor("xT", [D, T], MMDT, kind="ExternalInput").ap()
    wqT = nc.dram_tensor("wqT", [D, GD], MMDT, kind="ExternalInput").ap()
    wkvT = nc.dram_tensor("wkvT", [D, 2 * GKD], MMDT, kind="ExternalInput").ap()
    wpT = nc.dram_tensor("wpT", [GD, D], MMDT, kind="ExternalInput").ap()
    cosT = nc.dram_tensor("cosT", [T, HD], F32, kind="ExternalInput").ap()
    sinT = nc.dram_tensor("sinT", [T, HD], F32, kind="ExternalInput").ap()
    g2t = nc.dram_tensor("g2t", [P, NH], F32, kind="ExternalInput").ap()
    out = nc.dram_tensor("out", [T, D], F32, kind="ExternalOutput").ap()

    xT_v = xT.rearrange("(ko p) t -> p ko t", p=P)
    wqT_v = wqT.rearrange("(ko p) m -> p ko m", p=P)
    wkvT_v = wkvT.rearrange("(ko p) m -> p ko m", p=P)
    wpT_v = wpT.rearrange("(ko p) m -> p ko m", p=P)

    with tile.TileContext(nc) as tc:
        with (
            tc.tile_pool(name="const", bufs=1) as constp,
            tc.tile_pool(name="resident", bufs=1) as respool,
        ):
            ident = constp.tile([P, P], MMDT)
            make_identity(nc, ident)
            ones_col = constp.tile([P, 1], MMDT)   # rhs for transposed-l mms
            nc.vector.memset(ones_col[:], 1.0)
            ones_row = constp.tile([1, P], MMDT)   # lhsT for the lT zero-init
            nc.vector.memset(ones_row[:], 1.0)
            zrow = constp.tile([1, 4 * NJ], MMDT)  # rhs for the lT zero-init
            nc.vector.memset(zrow[:], 0.0)
            epsB = constp.tile([P, NH], F32)       # rmsnorm eps tile
            nc.vector.memset(epsB[:], EPS)
            zero_col = constp.tile([P, 1], F32)    # zero bias for Exp
            nc.vector.memset(zero_col[:], 0.0)
            g2 = constp.tile([P, NH], F32)         # per-head gain^2
            nc.sync.dma_start(g2[:], g2t)

            qT_all = respool.tile([P, GH, T], MMDT)
            kT_all = respool.tile([P, GKV, T], MMDT)
            v_all = respool.tile([P, NT, GKV, HD], MMDT)
            wp_sb = respool.tile([P, GH, D], MMDT)

            # ================= Phase A: QKV + norm + rope + transpose ====
            with (
                tc.tile_pool(name="wqkv", bufs=1) as wpool,
                tc.tile_pool(name="pa_sb", bufs=2) as sb,
                tc.tile_pool(name="pa_ps", bufs=2, space="PSUM") as ps,
                tc.tile_pool(name="pa_ps1", bufs=1, space="PSUM") as ps1a,
            ):
                xt0 = sb.tile([P, NKC, P], MMDT, tag="xt")
                # first k-chunk separately: the first matmul gates on 32KB,
                # not the full tile, while the 6MB weight preload streams
                nc.sync.dma_start(xt0[:, 0:1, :], xT_v[:, 0:1, bass.ts(0, P)])
                nc.sync.dma_start(xt0[:, 1:NKC, :],
                                  xT_v[:, 1:NKC, bass.ts(0, P)])
                cq0 = sb.tile([P, HD], F32, tag="cq")
                nc.sync.dma_start(cq0[:], cosT[bass.ts(0, P), :])
                sq0 = sb.tile([P, HD], F32, tag="sq")
                nc.sync.dma_start(sq0[:], sinT[bass.ts(0, P), :])
                wq_sb = wpool.tile([P, NKC, GD], MMDT)
                wkv_sb = wpool.tile([P, NKC, 2 * GKD], MMDT)
                for kc in range(NKC):
                    nc.sync.dma_start(wq_sb[:, kc, :], wqT_v[:, kc, :])
                    nc.sync.dma_start(wkv_sb[:, kc, :], wkvT_v[:, kc, :])

                for i in range(NT):
                    tsl = bass.ts(i, P)
                    if i == 0:
                        xt, cq, sq = xt0, cq0, sq0
                    else:
                        xt = sb.tile([P, NKC, P], MMDT, tag="xt")
                        nc.sync.dma_start(xt[:], xT_v[:, :, tsl])
                        cq = sb.tile([P, HD], F32, tag="cq")
                        nc.sync.dma_start(cq[:], cosT[tsl, :])
                        sq = sb.tile([P, HD], F32, tag="sq")
                        nc.sync.dma_start(sq[:], sinT[tsl, :])
                    if i >= 8:
                        # stream Wproj during the tail of phase A (after the
                        # front-loaded weight DMAs have drained)
                        nc.sync.dma_start(wp_sb[:, i - 8, :], wpT_v[:, i - 8, :])

                    q_ps = ps.tile([P, GD], F32, tag="qps")
                    kv_ps = ps.tile([P, 2 * GKD], F32, tag="kvps")
                    k_ps = kv_ps[:, 0:GKD]
                    v_ps = kv_ps[:, GKD:2 * GKD]
                    for kc in range(NKC):
                        st, sp = kc == 0, kc == NKC - 1
                        lx = _mm_ap(xt[:, kc, :])
                        nc.tensor.matmul(q_ps[:, 0:512], lx,
                                         _mm_ap(wq_sb[:, kc, 0:512]),
                                         start=st, stop=sp)
                        nc.tensor.matmul(q_ps[:, 512:1024], lx,
                                         _mm_ap(wq_sb[:, kc, 512:1024]),
                                         start=st, stop=sp)
                        nc.tensor.matmul(kv_ps[:], lx,
                                         _mm_ap(wkv_sb[:, kc, :]),
                                         start=st, stop=sp)

                    # ---- sum of squares per head (ScalarE), then rstd =
                    # g * (ssq/HD + eps)^-1/2 via DVE reciprocal + ACT Sqrt
                    # (Square and Sqrt live in one activation table set).
                    ssq = sb.tile([P, NH], F32, tag="ssq")
                    scr = sb.tile([P, P], F32, tag="scr")
                    for h in range(GH):
                        nc.scalar.activation(scr[:], q_ps[:, h * HD:(h + 1) * HD],
                                             AF.Square,
                                             accum_out=ssq[:, h:h + 1])
                    for h in range(GKV):
                        nc.scalar.activation(scr[:], k_ps[:, h * HD:(h + 1) * HD],
                                             AF.Square,
                                             accum_out=ssq[:, GH + h:GH + h + 1])
                    u = sb.tile([P, NH], F32, tag="u")
                    nc.vector.scalar_tensor_tensor(u[:], ssq[:], 1.0 / HD,
                                                   epsB[:], ALU.mult, ALU.add)
                    w = sb.tile([P, NH], F32, tag="w")
                    nc.vector.reciprocal(w[:], u[:])
                    v = sb.tile([P, NH], F32, tag="v")
                    nc.vector.tensor_tensor(v[:], w[:], g2[:], ALU.mult)
                    rstd = sb.tile([P, NH], F32, tag="rstd")
                    nc.scalar.activation(rstd[:], v[:], AF.Sqrt)

                    # ---- rope: (z*cos + shift(z)*sinflip) * rstd ----
                    # cos/sinflip are a single [P, HD] tile broadcast across
                    # heads (q gain is folded into rstd via g2).
                    def rope(z_ps, rs, nh, tag):
                        wd = nh * HD
                        t1 = sb.tile([P, nh, HD], F32, tag=tag + "t1")
                        z3 = z_ps[:, :wd].rearrange("p (h d) -> p h d", h=nh)
                        cq3 = cq[:, None, :].to_broadcast((P, nh, HD))
                        nc.vector.tensor_tensor(t1[:], z3, cq3, ALU.mult)
                        t2 = sb.tile([P, nh, HD], F32, tag=tag + "t2")
                        sqlo = sq[:, None, 0:64].to_broadcast((P, nh, 64))
                        sqhi = sq[:, None, 64:128].to_broadcast((P, nh, 64))
                        nc.vector.tensor_tensor(t2[:, :, 0:64],
                                                z3[:, :, 64:128], sqlo,
                                                ALU.mult)
                        nc.vector.tensor_tensor(t2[:, :, 64:128],
                                                z3[:, :, 0:64], sqhi,
                                                ALU.mult)
                        nc.vector.tensor_tensor(t1[:], t1[:], t2[:], ALU.add)
                        zf = sb.tile([P, nh, HD], MMDT, tag=tag + "zf")
                        nc.vector.tensor_tensor(
                            zf[:], t1[:],
                            rs[:, :, None].to_broadcast((P, nh, HD)), ALU.mult)
                        return zf

                    qf = rope(q_ps, rstd[:, 0:GH], GH, "q")
                    kf = rope(k_ps, rstd[:, GH:NH], GKV, "k")

                    # ---- PE transpose to [HD, T] layout, SBUF-resident.
                    tp = ps1a.tile([P, NH, P], MMDT, tag="tp")
                    for h in range(GH):
                        nc.tensor.transpose(tp[:, h, :], qf[:, h, :],
                                            ident[:])
                    for h in range(GKV):
                        nc.tensor.transpose(tp[:, GH + h, :], kf[:, h, :],
                                            ident[:])
                    nc.vector.tensor_copy(qT_all[:, :, tsl], tp[:, 0:GH, :])
                    nc.vector.tensor_copy(kT_all[:, :, tsl],
                                          tp[:, GH:NH, :])
                    nc.vector.tensor_copy(
                        v_all[:, i, :, :],
                        v_ps.rearrange("p (h d) -> p h d", h=GKV))

            # ================= Phase B: attention ========================
            with (
                tc.tile_pool(name="yall", bufs=1) as ypool,
                tc.tile_pool(name="pb_sb", bufs=2) as sb,
                tc.tile_pool(name="pb_pt", bufs=4) as ptp,
            ):
                yT_all = ypool.tile([P, GH, T], MMDT)
                with (
                    tc.tile_pool(name="pb_ps", bufs=2, space="PSUM") as ps,
                    tc.tile_pool(name="pb_ps1", bufs=2, space="PSUM") as ps1,
                    tc.tile_pool(name="pb_lt", bufs=2, space="PSUM") as pslt,
                    tc.tile_pool(name="pc_ps", bufs=2, space="PSUM") as cps,
                ):
                 def proj_tile(i):
                    # interleaved output projection for one 128-row tile,
                    # four 512-wide quarters (one PSUM bank each, 2 bufs)
                    tsl = bass.ts(i, P)
                    for nh in range(4):
                        o_ps = cps.tile([P, D // 4], F32, tag="ops")
                        for kc in range(GH):
                            nc.tensor.matmul(o_ps[:],
                                             _mm_ap(yT_all[:, kc, tsl]),
                                             _mm_ap(wp_sb[:, kc,
                                                    bass.ts(nh, 512)]),
                                             start=kc == 0, stop=kc == GH - 1)
                        o_sb = sb.tile([P, D // 4], F32, tag="osb")
                        nc.vector.tensor_copy(o_sb[:], o_ps[:])
                        nc.sync.dma_start(
                            out[tsl, bass.ts(nh, 512)], o_sb[:])

                 def emit_tail(bt, kht, lT_ps, y4, bsl_t):
                    # 1/l for the 4 heads x 4 x-chunks of this (b, kh) group:
                    # DVE reciprocal of the [x, 1] columns, single-column PE
                    # transposes land the rows on partition 0, and gpsimd
                    # partition_broadcast spreads each row across partitions
                    # for the y^T normalization multiply.
                    linv = sb.tile([P, 4 * NJ], F32, tag="linv")
                    nc.vector.reciprocal(linv[:], lT_ps[:, 0:4 * NJ])
                    linvb = sb.tile([P, 4 * NJ], MMDT, tag="linvb")
                    nc.vector.tensor_copy(linvb[:], linv[:])
                    rows = lT_ps[0:1, 4 * NJ:].bitcast(MMDT)
                    for hi, h in enumerate(range(kht * 4, kht * 4 + 4)):
                        for j in range(NJ):
                            nc.tensor.transpose(
                                rows[0:1, j * P:(j + 1) * P],
                                linvb[:, hi * NJ + j:hi * NJ + j + 1],
                                ident[:])
                        row_sb = sb.tile([1, QBLK], MMDT, tag="rowsb")
                        nc.vector.tensor_copy(row_sb[:], rows[0:1, :])
                        li_sb = sb.tile([P, QBLK], MMDT, tag="lisb")
                        nc.gpsimd.partition_broadcast(li_sb[:],
                                                      row_sb[0:1, :],
                                                      channels=P)
                        nc.vector.tensor_tensor(yT_all[:, h, bsl_t],
                                                y4[:, hi, :],
                                                li_sb[:], ALU.mult)

                 pending = None
                 pending_proj = None
                 for b in range(NB):
                    nch = (b + 1) * NJ
                    bsl = bass.ds(b * QBLK, QBLK)
                    for kh in range(GKV):
                        kt_blk = kT_all[:, kh, :]
                        # cols 0:16 hold the transposed-l sums; the f32
                        # cols 16:272 of partition 0 are reused (bitcast to
                        # bf16) as the 1/l row staging area in emit_tail, so
                        # rows travel with the same deferred tile generation
                        lT_ps = pslt.tile([P, 4 * NJ + 4 * QBLK // 8], F32,
                                          tag="ltps")
                        # first_mm clears the WHOLE bank's has_written bits,
                        # so interleaved column accumulation groups must not
                        # each carry start=True: one zero-writing init mm
                        # marks all 16 columns, the l sums accumulate onto it
                        nc.tensor.matmul(lT_ps[:, 0:4 * NJ],
                                         _mm_ap(ones_row[:]), _mm_ap(zrow[:]),
                                         start=True, stop=True)
                        y4 = sb.tile([P, 4, QBLK], F32, tag="y4")
                        for hi, h in enumerate(range(kh * 4, kh * 4 + 4)):
                            qt_blk = qT_all[:, h, bsl]
                            y_ps = ps1.tile([P, QBLK], F32, tag="yps")
                            for c in range(nch):
                                # columns x < x0 of this chunk are fully
                                # masked by causality; skip them entirely
                                x0 = max(0, (c - NJ * b) * P)
                                wd = QBLK - x0
                                st_ps = ps.tile([P, QBLK], F32, tag="stps")
                                nc.tensor.matmul(
                                    st_ps[:, x0:QBLK],
                                    _mm_ap(kt_blk[:, c * P:(c + 1) * P]),
                                    _mm_ap(qt_blk[:, x0:QBLK]),
                                    start=True, stop=True)
                                pt = ptp.tile([P, QBLK], MMDT, tag="pt")
                                nc.scalar.activation(pt[:, x0:QBLK],
                                                     st_ps[:, x0:QBLK],
                                                     AF.Exp, scale=SCALE,
                                                     bias=zero_col[:])
                                if c >= NJ * b:
                                    # triangular strip: keep where x - p >= 0
                                    nc.gpsimd.affine_select(
                                        out=pt[:, x0:x0 + P],
                                        in_=pt[:, x0:x0 + P],
                                        compare_op=ALU.is_ge, fill=0.0,
                                        base=0, channel_multiplier=-1,
                                        pattern=[[1, P]])
                                stt, spp = c == 0, c == nch - 1
                                # transposed softmax-denominator matmuls:
                                # pt chunk stationary, 1-wide ones moving
                                for j in range(x0 // P, NJ):
                                    nc.tensor.matmul(
                                        lT_ps[:, hi * NJ + j:hi * NJ + j + 1],
                                        _mm_ap(pt[:, j * P:(j + 1) * P]),
                                        _mm_ap(ones_col[:]),
                                        start=False,
                                        stop=c == NJ * b + j)
                                nc.tensor.matmul(y_ps[:, x0:QBLK],
                                                 _mm_ap(v_all[:, c, kh, :]),
                                                 _mm_ap(pt[:, x0:QBLK]),
                                                 start=stt, stop=spp)
                            nc.vector.tensor_copy(y4[:, hi, :], y_ps[:])
                        if pending_proj is not None:
                            for i in range(4 * pending_proj,
                                           4 * pending_proj + 4):
                                proj_tile(i)
                            pending_proj = None
                        if pending is not None:
                            emit_tail(*pending)
                            if pending[1] == 1:
                                pending_proj = pending[0]
                        pending = (b, kh, lT_ps, y4, bsl)

                 if pending_proj is not None:
                     for i in range(4 * pending_proj, 4 * pending_proj + 4):
                         proj_tile(i)
                 if pending is not None:
                     emit_tail(*pending)
                     if pending[1] == 1:
                         for i in range(4 * pending[0], 4 * pending[0] + 4):
                             proj_tile(i)

    nc.compile()
    return nc


def _np_mmdt():
    if MM_MODE == "bf16":
        import ml_dtypes
        return ml_dtypes.bfloat16
    return np.float32


def _host_prep(x, Wq, Wk, Wv, Wproj, q_gain):
    """Build the 8 per-core input maps."""
    mdt = _np_mmdt()
    t = np.arange(T, dtype=np.float64)
    inv_freq = 1.0 / (BASE ** (np.arange(0, HD, 2, dtype=np.float64) / HD))
    freqs = np.outer(t, inv_freq)
    emb = np.concatenate([freqs, freqs], axis=-1)
    cos = np.ascontiguousarray(np.cos(emb).astype(np.float32))
    sin = np.sin(emb).astype(np.float32)
    sinflip = np.ascontiguousarray(
        np.concatenate([-sin[:, :64], sin[:, :64]], axis=-1))

    in_maps = []
    for c in range(NCORES):
        b, g = c // 2, c % 2
        gain = q_gain[g * GH:(g + 1) * GH].astype(np.float32)
        g2 = np.concatenate([gain * gain, np.ones(GKV, np.float32)])
        g2t = np.ascontiguousarray(np.broadcast_to(g2[None, :], (P, NH)))
        in_maps.append({
            "xT": np.ascontiguousarray(x[b].T).astype(mdt),
            "wqT": np.ascontiguousarray(Wq[g * GD:(g + 1) * GD, :].T).astype(mdt),
            "wkvT": np.ascontiguousarray(np.concatenate(
                [Wk[g * GKD:(g + 1) * GKD, :].T,
                 Wv[g * GKD:(g + 1) * GKD, :].T], axis=1)).astype(mdt),
            "wpT": np.ascontiguousarray(Wproj[:, g * GD:(g + 1) * GD].T).astype(mdt),
            "cosT": cos,
            "sinT": sinflip,
            "g2t": g2t,
        })
    return in_maps


def run(x, Wq, Wk, Wv, Wproj, q_gain, trace=False):
    if "nc" not in _CACHE:
        _CACHE["nc"] = _build_program()
    nc = _CACHE["nc"]
    in_maps = _host_prep(
        np.asarray(x, np.float32), np.asarray(Wq, np.float32),
        np.asarray(Wk, np.float32), np.asarray(Wv, np.float32),
        np.asarray(Wproj, np.float32), np.asarray(q_gain, np.float32))
    try:
        res = run_bass_kernel_spmd(nc, in_maps, list(range(NCORES)),
                                   trace=trace)
    except ModuleNotFoundError:
        res = run_bass_kernel_spmd(nc, in_maps, list(range(NCORES)),
                                   trace=False)
    outs = np.zeros((B, T, D), np.float32)
    for c in range(NCORES):
        outs[c // 2] += res.results[c]["out"]
    return outs, res.exec_time_ns


def kernel(**inputs):
    out, _ = run(inputs["x"], inputs["Wq"], inputs["Wk"], inputs["Wv"],
                 inputs["Wproj"], inputs["q_gain"])
    return out
